# revision 9
# baseline (speedup 1.0000x reference)
"""CapsuleLayer (dynamic routing) Trainium2 kernel, 8-core SPMD — v3 (fp16).

Sharding: n_in (2048) split 8 ways -> 256 rows/core; the only cross-core data
is the [b, c, e] routing sum `s`, AllReduced once per iteration (3x 128KB fp16).

Measured-on-hw design notes:
  * fp8/DoubleRow matmuls are numerically out (e4m3 x/W quantization alone is
    4e-2 rel err vs the 2e-2 gate), so u matmuls are fp16, 1 col/cycle.
  * DVE: plain tensor_tensor runs 2x_1p (0.52 ns/elem fp16); the fancy
    scalar_tensor_tensor form is 1x on real hw (cost model is wrong), so all
    big elementwise/tree ops are plain TT. tensor_scalar runs 4x.
  * The low tree levels + logit accumulation ride on the Pool engine
    (tensor_add; ~4 ns/elem but fully parallel), PSUM evacuation on ACT.
  * Softmax: one batched ACT exp (f32), Z via a single DVE tensor_reduce
    (instead of 4 ACT accumulator reads), c = eb * (1/Z)-broadcast.
  * Per-iteration AllReduce tail squashes directly on the partition-
    replicated AllReduce result; u production for the next pass runs through
    the tail (deep u2/t3 buffering + lagged sel matmuls).

Per-core layout: partition row p of [128, *] tensors is (j, b) = (p//32,
p%32). Free axis of u is e-major (col = e*64 + c). Pass A computes
s0_raw = sum_n u / 8 directly with collapsed x/8 weights (M=32, bias row
0.5), rescaled during the squash tail.
"""

import numpy as np
from contextlib import ExitStack

import concourse.bass as bass
import concourse.tile as tile
from concourse import mybir
from concourse.bass_utils import run_bass_kernel_spmd

F16 = mybir.dt.float16
F32 = mybir.dt.float32
AF = mybir.ActivationFunctionType
OP = mybir.AluOpType

N_CORES = 8
BT, NN, DD = 32, 2048, 16      # batch, n_in, d_in
CC, EE = 64, 32                # n_capsule, d_capsule
NL = NN // N_CORES             # 256 local n rows
G4 = 4                         # n rows per matmul group
NG = NL // G4                  # 64 groups
NQ = NG // 4                   # 16 quads (4 groups each)
KK = G4 * DD + 1               # 65 contraction rows (incl. bias row)
CE = CC * EE                   # 2048, e-major: col = e*CC + c
EPS = 1e-9
LAG = 4                        # sel-matmul lag, in pairs


def _split_waits(nc):
    """walrus CTRL codegen only supports one sem-wait per instruction; hoist
    extra waits into preceding NoOps on the same engine."""
    for f in nc.m.functions:
        for bb in f.blocks:
            new_insts = []
            for inst in bb.instructions:
                si = inst.sync_info
                if si is not None and si.on_wait and len(si.on_wait) > 1:
                    waits = list(si.on_wait)
                    for w in waits[:-1]:
                        new_insts.append(mybir.InstNoOp(
                            name=f"WS-{nc.next_id()}",
                            sync_info=mybir.SyncInfo(on_wait=[w], on_update=[]),
                            bass_nofuse=True,
                            engine=inst.engine,
                        ))
                    inst.sync_info = mybir.SyncInfo(
                        on_wait=waits[-1:], on_update=si.on_update)
                new_insts.append(inst)
            bb.instructions = new_insts


def _bcast(ap, n, axis_pos):
    """Insert a [step=0, count=n] dim into an AP at free-dim position axis_pos
    (0 = right after the partition dim)."""
    dims = [list(d) for d in ap.ap]
    dims.insert(1 + axis_pos, [0, n])
    return bass.AP(tensor=ap.tensor, offset=ap.offset, ap=dims)


def _build_program():
    nc = bass.Bass()
    xg = nc.declare_dram_parameter("xg", [KK, NG, 128], F16, isOutput=False)
    xcA = nc.declare_dram_parameter("xcA", [KK, NG, BT], F16, isOutput=False)
    wg = nc.declare_dram_parameter("wg", [NG, KK, CE], F16, isOutput=False)
    sel1 = nc.declare_dram_parameter("sel1", [128, BT], F16, isOutput=False)
    vout = nc.declare_dram_parameter("vout", [BT, CC, EE], F32, isOutput=True)

    with ExitStack() as ctx:
        tc = ctx.enter_context(tile.TileContext(nc))
        singles = ctx.enter_context(tc.tile_pool(name="singles", bufs=1))
        wpool = ctx.enter_context(tc.tile_pool(name="wpool", bufs=2))
        upool = ctx.enter_context(tc.tile_pool(name="upool", bufs=4))
        t1pool = ctx.enter_context(tc.tile_pool(name="t1pool", bufs=1))
        t3pool = ctx.enter_context(tc.tile_pool(name="t3pool", bufs=LAG + 2))
        trpool = ctx.enter_context(tc.tile_pool(name="trpool", bufs=1))
        smpool = ctx.enter_context(tc.tile_pool(name="smpool", bufs=2))
        tailp = ctx.enter_context(tc.tile_pool(name="tailp", bufs=1))
        psum_u = ctx.enter_context(tc.tile_pool(name="psum_u", bufs=2, space="PSUM"))
        psum_s = ctx.enter_context(tc.tile_pool(name="psum_s", bufs=1, space="PSUM"))
        dram = ctx.enter_context(tc.tile_pool(name="dram", bufs=1, space="DRAM"))

        xcA_sb = singles.tile([KK, NG, BT], F16)
        for h in range(2):
            nc.sync.dma_start(out=xcA_sb[:, 32 * h:32 * (h + 1), :],
                              in_=xcA[:, 32 * h:32 * (h + 1), :])
        sel1_sb = singles.tile([128, BT], F16)
        nc.sync.dma_start(out=sel1_sb[:], in_=sel1[:])
        xg_sb = singles.tile([KK, NG, 128], F16)

        bB = singles.tile([128, NG, CC], F16)
        vrep = [singles.tile([128, CE], F16, name=f"vrep{i}", tag=f"vrep{i}")
                for i in range(2)]
        epst = singles.tile([128, 1], F32)
        nc.vector.memset(epst[:], EPS)

        def dma_w_quad(q):
            wt = wpool.tile([KK, 4, CE], F16, tag="wt")
            src = wg[4 * q:4 * q + 4]  # [4, KK, CE]
            dims = [list(d) for d in src.ap]
            re = bass.AP(tensor=src.tensor, offset=src.offset,
                         ap=[dims[1], dims[0], dims[2]])
            nc.sync.dma_start(out=wt[:], in_=re)
            return wt

        def s_to_v(s_ps, it):
            """AllReduce s across cores, squash on the replicated result,
            build vrep[it] (or write vout for it==2).

            Pass A feeds s_raw = 8*s0 (x pre-scaled by 1/8), so it==0
            rescales: ns = ns_raw/64, scale *= 1/8."""
            ssb = tailp.tile([32, CE], F16, tag="ssb")
            nc.scalar.copy(ssb[:], s_ps[:])
            sloc = dram.tile([32, CE], F16, tag=f"sloc{it}")
            nc.sync.dma_start(out=sloc[:], in_=ssb[:])
            ssum = dram.tile([32, CE], F16, tag=f"ssum{it}")
            nc.gpsimd.collective_compute(
                "AllReduce", OP.add,
                replica_groups=[list(range(N_CORES))],
                ins=[sloc[:].opt()], outs=[ssum[:].opt()])
            srep = tailp.tile([128, CE], F16, tag="srep")
            sap = ssum[:]
            rep = bass.AP(tensor=sap.tensor, offset=sap.offset,
                          ap=[[0, 4]] + [list(d) for d in sap.ap])
            nc.sync.dma_start(out=srep[:], in_=rep)

            s2 = tailp.tile([128, CE], F16, tag="s2")
            nc.vector.tensor_mul(s2[:], srep[:], srep[:])
            n1 = tailp.tile([128, 1024], F16, tag="n1")
            nc.vector.tensor_add(n1[:], s2[:, 0:1024], s2[:, 1024:2048])
            n2 = tailp.tile([128, 512], F16, tag="n2")
            nc.vector.tensor_add(n2[:], n1[:, 0:512], n1[:, 512:1024])
            n3 = tailp.tile([128, 256], F16, tag="n3")
            nc.vector.tensor_add(n3[:], n2[:, 0:256], n2[:, 256:512])
            n4 = tailp.tile([128, 128], F16, tag="n4")
            nc.vector.tensor_add(n4[:], n3[:, 0:128], n3[:, 128:256])
            ns16 = tailp.tile([128, CC], F16, tag="ns16")
            nc.vector.tensor_add(ns16[:], n4[:, 0:64], n4[:, 64:128])

            k_ns = (1.0 / 64.0) if it == 0 else 1.0
            k_sc = 0.125 if it == 0 else 1.0
            ns = tailp.tile([128, CC], F32, tag="ns")
            nc.vector.tensor_scalar_mul(ns[:], ns16[:], k_ns)
            sq = tailp.tile([128, CC], F32, tag="sq")
            nc.scalar.activation(sq[:], ns[:], AF.Sqrt, bias=epst[:], scale=1.0)
            den = tailp.tile([128, CC], F32, tag="den")
            nc.vector.scalar_tensor_tensor(den[:], ns[:], 1.0, sq[:],
                                           op0=OP.add, op1=OP.mult)
            inv = tailp.tile([128, CC], F32, tag="inv")
            nc.vector.reciprocal(inv[:], den[:])
            sc = tailp.tile([128, CC], F16, tag="sc")
            nc.vector.scalar_tensor_tensor(sc[:], ns[:], k_sc, inv[:],
                                           op0=OP.mult, op1=OP.mult)

            if it < 2:
                vv = vrep[it][:].rearrange("p (e c) -> p e c", e=EE)
                sv = srep[:].rearrange("p (e c) -> p e c", e=EE)
                nc.vector.tensor_mul(vv, sv, _bcast(sc[:], EE, 0))
            else:
                vcm = tailp.tile([32, CE], F32, tag="vcm")
                vcm_t = bass.AP(
                    tensor=vcm[:].tensor, offset=vcm[:].offset,
                    ap=[list(vcm[:].ap[0]), [1, EE], [EE, CC]])
                sv = srep[0:32].rearrange("p (e c) -> p e c", e=EE)
                nc.vector.tensor_mul(vcm_t, sv, _bcast(sc[0:32], EE, 0))
                vcm_v = vcm[:].rearrange("p (c e) -> p c e", c=CC)
                nc.sync.dma_start(out=vout[0:16], in_=vcm_v[0:16])
                nc.sync.dma_start(out=vout[16:32], in_=vcm_v[16:32])

        # ---------------- pass A: s0_raw = sum_n u / 8 -----------------------
        sA = psum_s.tile([32, CE], F32, tag="s_ps")
        for q in range(NQ):
            wt = dma_w_quad(q)
            if q == 1:
                for h in range(4):  # xg (1MB) spread over 4 issues
                    nc.sync.dma_start(
                        out=xg_sb[:, 16 * h:16 * (h + 1), :],
                        in_=xg[:, 16 * h:16 * (h + 1), :])
            for gi in range(4):
                lhsT = xcA_sb[:, 4 * q + gi, :]
                for hh in range(4):
                    nc.tensor.matmul(
                        sA[:, hh * 512:(hh + 1) * 512],
                        lhsT,
                        wt[:, gi, hh * 512:(hh + 1) * 512],
                        start=(q == 0 and gi == 0),
                        stop=(q == NQ - 1 and gi == 3))
        s_to_v(sA, 0)

        # ---------------- passes B (it=1) and C (it=2) -----------------------
        for it in (1, 2):
            sP = psum_s.tile([32, CE], F32, tag="s_ps")
            vr = vrep[it - 1]
            t3q = []

            def flush_sel(ent, sP=sP):
                t3t, pair = ent
                for gg in range(2):
                    for hh in range(4):
                        nc.tensor.matmul(
                            sP[:, hh * 512:(hh + 1) * 512],
                            sel1_sb[:],
                            t3t[:, gg, hh * 512:(hh + 1) * 512],
                            start=(pair == 0 and gg == 0),
                            stop=(pair == 2 * NQ - 1 and gg == 1))

            for q in range(NQ):
                wt = dma_w_quad(q)
                u2p = [upool.tile([128, 2, CE], F16, name=f"u2_{pp}", tag="u2")
                       for pp in range(2)]
                for gi in range(4):
                    for h in range(2):
                        ups = psum_u.tile([128, 1024], F32, tag="ups")
                        for hh in range(2):
                            nc.tensor.matmul(
                                ups[:, hh * 512:(hh + 1) * 512],
                                xg_sb[:, 4 * q + gi, :],
                                wt[:, gi,
                                   h * 1024 + hh * 512:h * 1024 + (hh + 1) * 512],
                                start=True, stop=True)
                        dst = u2p[gi // 2][:, gi % 2, h * 1024:(h + 1) * 1024]
                        nc.scalar.copy(dst, ups[:])

                # db[p, g, c] = sum_e u*v: TT mult (2x) + tree; low levels on
                # the Pool engine to keep DVE for the big ops
                r1 = trpool.tile([128, 4, 1024], F16, tag="r1")
                for p in range(2):
                    t1 = t1pool.tile([128, 2, CE], F16, tag="t1")
                    nc.vector.tensor_mul(t1[:], u2p[p][:], _bcast(vr[:], 2, 0))
                    nc.vector.tensor_add(r1[:, 2 * p:2 * p + 2, :],
                                         t1[:, :, 0:1024], t1[:, :, 1024:2048])
                r2 = trpool.tile([128, 4, 512], F16, tag="r2")
                nc.vector.tensor_add(r2[:], r1[:, :, 0:512], r1[:, :, 512:1024])
                r3 = trpool.tile([128, 4, 256], F16, tag="r3")
                nc.gpsimd.tensor_add(r3[:], r2[:, :, 0:256], r2[:, :, 256:512])
                r4 = trpool.tile([128, 4, 128], F16, tag="r4")
                nc.gpsimd.tensor_add(r4[:], r3[:, :, 0:128], r3[:, :, 128:256])
                if it == 1:
                    nc.gpsimd.tensor_add(bB[:, 4 * q:4 * q + 4, :],
                                         r4[:, :, 0:64], r4[:, :, 64:128])
                    blog = bB[:, 4 * q:4 * q + 4, :]
                else:
                    bt = smpool.tile([128, 4, CC], F16, tag="bt")
                    nc.gpsimd.tensor_add(bt[:], r4[:, :, 0:64], r4[:, :, 64:128])
                    bt2 = smpool.tile([128, 4, CC], F16, tag="bt2")
                    nc.gpsimd.tensor_add(bt2[:], bt[:], bB[:, 4 * q:4 * q + 4, :])
                    blog = bt2[:]

                # softmax over c: one batched exp, Z per group via one DVE
                # reduce, c = eb * (1/Z) broadcast
                eb = smpool.tile([128, 4, CC], F32, tag="eb")
                nc.scalar.activation(eb[:], blog[:], AF.Exp)
                zz = smpool.tile([128, 4], F32, tag="zz")
                nc.vector.tensor_reduce(zz[:], eb[:],
                                        axis=mybir.AxisListType.X, op=OP.add)
                iz = smpool.tile([128, 4], F32, tag="iz")
                nc.vector.reciprocal(iz[:], zz[:])
                cc = smpool.tile([128, 4, CC], F16, tag="cc")
                nc.vector.tensor_mul(cc[:], eb[:], _bcast(iz[:], CC, 1))

                # t3 = c*u, queued for the (lagged) sel matmuls
                for p in range(2):
                    t3 = t3pool.tile([128, 2, CE], F16, tag="t3")
                    t3v = t3[:].rearrange("p g (e c) -> p g e c", e=EE)
                    u2v = u2p[p][:].rearrange("p g (e c) -> p g e c", e=EE)
                    ccp = cc[:, 2 * p:2 * p + 2, :]
                    nc.vector.tensor_mul(t3v, u2v, _bcast(ccp, EE, 1))
                    t3q.append((t3, 2 * q + p))
                    if len(t3q) > LAG:
                        flush_sel(t3q.pop(0))
            while t3q:
                flush_sel(t3q.pop(0))
            s_to_v(sP, it)

    _split_waits(nc)
    return nc


_CACHE = {}


def _prep_inputs(x, W, B):
    """Host-side fp16 layout prep: block-diagonal x tiles (ones bias row),
    collapsed x/8 for pass A (bias row 0.5), W rows=(j,d | bias=B) cols=(e,c)."""
    x = np.asarray(x, np.float32)
    W = np.asarray(W, np.float32)
    Bm = np.asarray(B, np.float32)

    xr = x.transpose(1, 2, 0).reshape(N_CORES, NG, G4, DD, BT)  # [c,g,j,d,b]
    xg = np.zeros((N_CORES, KK, NG, 128), np.float16)
    xcA = np.zeros((N_CORES, KK, NG, BT), np.float16)
    for j in range(G4):
        blk = xr[:, :, j].transpose(0, 2, 1, 3)  # [core, d, g, b]
        xg[:, j * DD:(j + 1) * DD, :, j * BT:(j + 1) * BT] = blk
        xcA[:, j * DD:(j + 1) * DD, :, :] = blk / 8.0
    xg[:, G4 * DD, :, :] = 1.0
    xcA[:, G4 * DD, :, :] = 0.5

    Wr = W.reshape(N_CORES, NG, G4, CC, DD, EE).transpose(0, 1, 2, 4, 5, 3)
    wg = np.zeros((N_CORES, NG, KK, CE), np.float16)
    wg[:, :, :G4 * DD, :] = Wr.reshape(N_CORES, NG, G4 * DD, CE)
    wg[:, :, G4 * DD, :] = Bm.T.reshape(CE).astype(np.float16)

    sel1 = np.zeros((128, BT), np.float16)
    for p in range(128):
        sel1[p, p % BT] = 1.0
    return xg, xcA, wg, sel1


def _in_maps(x, W, B):
    xg, xcA, wg, sel1 = _prep_inputs(x, W, B)
    return [
        {"xg": np.ascontiguousarray(xg[k]),
         "xcA": np.ascontiguousarray(xcA[k]),
         "wg": np.ascontiguousarray(wg[k]),
         "sel1": sel1}
        for k in range(N_CORES)
    ]


def kernel(x, W, B):
    if "nc" not in _CACHE:
        _CACHE["nc"] = _build_program()
    nc = _CACHE["nc"]
    res = run_bass_kernel_spmd(nc, _in_maps(x, W, B), list(range(N_CORES)))
    return np.asarray(res.results[0]["vout"], np.float32)


# revision 12
# speedup vs baseline: 1.0645x; 1.0645x over previous
"""CapsuleLayer (dynamic routing) Trainium2 kernel, 8-core SPMD — v3 (fp16).

Sharding: n_in (2048) split 8 ways -> 256 rows/core; the only cross-core data
is the [b, c, e] routing sum `s`, AllReduced once per iteration (3x 128KB fp16).

Measured-on-hw design notes:
  * fp8/DoubleRow matmuls are numerically out (e4m3 x/W quantization alone is
    4e-2 rel err vs the 2e-2 gate), so u matmuls are fp16, 1 col/cycle.
  * DVE: plain tensor_tensor runs 2x_1p (0.52 ns/elem fp16); the fancy
    scalar_tensor_tensor form is 1x on real hw (cost model is wrong), so all
    big elementwise/tree ops are plain TT. tensor_scalar runs 4x.
  * The low tree levels + logit accumulation ride on the Pool engine
    (tensor_add; ~4 ns/elem but fully parallel), PSUM evacuation on ACT.
  * Softmax: one batched ACT exp (f32), Z via a single DVE tensor_reduce
    (instead of 4 ACT accumulator reads), c = eb * (1/Z)-broadcast.
  * Per-iteration AllReduce tail squashes directly on the partition-
    replicated AllReduce result; u production for the next pass runs through
    the tail (deep u2/t3 buffering + lagged sel matmuls).

Per-core layout: partition row p of [128, *] tensors is (j, b) = (p//32,
p%32). Free axis of u is e-major (col = e*64 + c). Pass A computes
s0_raw = sum_n u / 8 directly with collapsed x/8 weights (M=32, bias row
0.5), rescaled during the squash tail.
"""

import numpy as np
from contextlib import ExitStack

import concourse.bass as bass
import concourse.tile as tile
from concourse import mybir
from concourse.bass_utils import run_bass_kernel_spmd

F16 = mybir.dt.float16
F32 = mybir.dt.float32
AF = mybir.ActivationFunctionType
OP = mybir.AluOpType

N_CORES = 8
BT, NN, DD = 32, 2048, 16      # batch, n_in, d_in
CC, EE = 64, 32                # n_capsule, d_capsule
NL = NN // N_CORES             # 256 local n rows
G4 = 4                         # n rows per matmul group
NG = NL // G4                  # 64 groups
NQ = NG // 4                   # 16 quads (4 groups each)
KK = G4 * DD + 1               # 65 contraction rows (incl. bias row)
CE = CC * EE                   # 2048, e-major: col = e*CC + c
EPS = 1e-9
LAG = 4                        # sel-matmul lag, in pairs


def _split_waits(nc):
    """walrus CTRL codegen only supports one sem-wait per instruction; hoist
    extra waits into preceding NoOps on the same engine."""
    for f in nc.m.functions:
        for bb in f.blocks:
            new_insts = []
            for inst in bb.instructions:
                si = inst.sync_info
                if si is not None and si.on_wait and len(si.on_wait) > 1:
                    waits = list(si.on_wait)
                    for w in waits[:-1]:
                        new_insts.append(mybir.InstNoOp(
                            name=f"WS-{nc.next_id()}",
                            sync_info=mybir.SyncInfo(on_wait=[w], on_update=[]),
                            bass_nofuse=True,
                            engine=inst.engine,
                        ))
                    inst.sync_info = mybir.SyncInfo(
                        on_wait=waits[-1:], on_update=si.on_update)
                new_insts.append(inst)
            bb.instructions = new_insts


def _bcast(ap, n, axis_pos):
    """Insert a [step=0, count=n] dim into an AP at free-dim position axis_pos
    (0 = right after the partition dim)."""
    dims = [list(d) for d in ap.ap]
    dims.insert(1 + axis_pos, [0, n])
    return bass.AP(tensor=ap.tensor, offset=ap.offset, ap=dims)


def _build_program():
    nc = bass.Bass()
    xg = nc.declare_dram_parameter("xg", [KK, NG, 128], F16, isOutput=False)
    xcA = nc.declare_dram_parameter("xcA", [KK, NG, BT], F16, isOutput=False)
    wg = nc.declare_dram_parameter("wg", [NG, KK, CE], F16, isOutput=False)
    sel1 = nc.declare_dram_parameter("sel1", [128, BT], F16, isOutput=False)
    vout = nc.declare_dram_parameter("vout", [BT, CC, EE], F32, isOutput=True)

    with ExitStack() as ctx:
        tc = ctx.enter_context(tile.TileContext(nc))
        singles = ctx.enter_context(tc.tile_pool(name="singles", bufs=1))
        wpool = ctx.enter_context(tc.tile_pool(name="wpool", bufs=2))
        upool = ctx.enter_context(tc.tile_pool(name="upool", bufs=5))
        t1pool = ctx.enter_context(tc.tile_pool(name="t1pool", bufs=1))
        t3pool = ctx.enter_context(tc.tile_pool(name="t3pool", bufs=LAG + 1))
        trpool = ctx.enter_context(tc.tile_pool(name="trpool", bufs=2))
        trpool1 = ctx.enter_context(tc.tile_pool(name="trpool1", bufs=1))
        smpool = ctx.enter_context(tc.tile_pool(name="smpool", bufs=2))
        tailp = ctx.enter_context(tc.tile_pool(name="tailp", bufs=1))
        psum_u = ctx.enter_context(tc.tile_pool(name="psum_u", bufs=2, space="PSUM"))
        psum_s = ctx.enter_context(tc.tile_pool(name="psum_s", bufs=1, space="PSUM"))
        dram = ctx.enter_context(tc.tile_pool(name="dram", bufs=1, space="DRAM"))

        xcA_sb = singles.tile([KK, NG, BT], F16)
        for h in range(2):
            nc.sync.dma_start(out=xcA_sb[:, 32 * h:32 * (h + 1), :],
                              in_=xcA[:, 32 * h:32 * (h + 1), :])
        sel1_sb = singles.tile([128, BT], F16)
        nc.sync.dma_start(out=sel1_sb[:], in_=sel1[:])
        xg_sb = singles.tile([KK, NG, 128], F16)

        bB = singles.tile([128, NG, CC], F16)
        vrep_t = singles.tile([128, CE], F16, name="vrep", tag="vrep")
        vrep = [vrep_t, vrep_t]
        epst = singles.tile([128, 1], F32)
        nc.vector.memset(epst[:], EPS)

        def dma_w_quad(q):
            wt = wpool.tile([KK, 4, CE], F16, tag="wt")
            src = wg[4 * q:4 * q + 4]  # [4, KK, CE]
            dims = [list(d) for d in src.ap]
            re = bass.AP(tensor=src.tensor, offset=src.offset,
                         ap=[dims[1], dims[0], dims[2]])
            nc.sync.dma_start(out=wt[:], in_=re)
            return wt

        def s_to_v(s_ps, it):
            """AllReduce s across cores, squash on the replicated result,
            build vrep[it] (or write vout for it==2).

            Pass A feeds s_raw = 8*s0 (x pre-scaled by 1/8), so it==0
            rescales: ns = ns_raw/64, scale *= 1/8."""
            ssb = tailp.tile([32, CE], F16, tag="ssb")
            nc.scalar.copy(ssb[:], s_ps[:])
            sloc = dram.tile([32, CE], F16, tag=f"sloc{it}")
            nc.scalar.dma_start(out=sloc[:], in_=ssb[:])
            ssum = dram.tile([32, CE], F16, tag=f"ssum{it}")
            nc.gpsimd.collective_compute(
                "AllReduce", OP.add,
                replica_groups=[list(range(N_CORES))],
                ins=[sloc[:].opt()], outs=[ssum[:].opt()])
            srep = tailp.tile([128, CE], F16, tag="srep")
            sap = ssum[:]
            rep = bass.AP(tensor=sap.tensor, offset=sap.offset,
                          ap=[[0, 4]] + [list(d) for d in sap.ap])
            nc.scalar.dma_start(out=srep[:], in_=rep)

            s2 = tailp.tile([128, CE], F16, tag="s2")
            nc.vector.tensor_mul(s2[:], srep[:], srep[:])
            n1 = tailp.tile([128, 1024], F16, tag="n1")
            nc.vector.tensor_add(n1[:], s2[:, 0:1024], s2[:, 1024:2048])
            n2 = tailp.tile([128, 512], F16, tag="n2")
            nc.vector.tensor_add(n2[:], n1[:, 0:512], n1[:, 512:1024])
            n3 = tailp.tile([128, 256], F16, tag="n3")
            nc.vector.tensor_add(n3[:], n2[:, 0:256], n2[:, 256:512])
            n4 = tailp.tile([128, 128], F16, tag="n4")
            nc.vector.tensor_add(n4[:], n3[:, 0:128], n3[:, 128:256])
            ns16 = tailp.tile([128, CC], F16, tag="ns16")
            nc.vector.tensor_add(ns16[:], n4[:, 0:64], n4[:, 64:128])

            k_ns = (1.0 / 64.0) if it == 0 else 1.0
            k_sc = 0.125 if it == 0 else 1.0
            ns = tailp.tile([128, CC], F32, tag="ns")
            nc.vector.tensor_scalar_mul(ns[:], ns16[:], k_ns)
            sq = tailp.tile([128, CC], F32, tag="sq")
            nc.scalar.activation(sq[:], ns[:], AF.Sqrt, bias=epst[:], scale=1.0)
            den = tailp.tile([128, CC], F32, tag="den")
            nc.vector.scalar_tensor_tensor(den[:], ns[:], 1.0, sq[:],
                                           op0=OP.add, op1=OP.mult)
            inv = tailp.tile([128, CC], F32, tag="inv")
            nc.vector.reciprocal(inv[:], den[:])
            sc = tailp.tile([128, CC], F16, tag="sc")
            nc.vector.scalar_tensor_tensor(sc[:], ns[:], k_sc, inv[:],
                                           op0=OP.mult, op1=OP.mult)

            if it < 2:
                vv = vrep[it][:].rearrange("p (e c) -> p e c", e=EE)
                sv = srep[:].rearrange("p (e c) -> p e c", e=EE)
                nc.vector.tensor_mul(vv, sv, _bcast(sc[:], EE, 0))
            else:
                vcm = tailp.tile([32, CE], F16, tag="vcm")
                vcm_t = bass.AP(
                    tensor=vcm[:].tensor, offset=vcm[:].offset,
                    ap=[list(vcm[:].ap[0]), [1, EE], [EE, CC]])
                sv = srep[0:32].rearrange("p (e c) -> p e c", e=EE)
                nc.vector.tensor_mul(vcm_t, sv, _bcast(sc[0:32], EE, 0))
                vcm_v = vcm[:].rearrange("p (c e) -> p c e", c=CC)
                nc.gpsimd.dma_start(out=vout[0:16], in_=vcm_v[0:16])
                nc.gpsimd.dma_start(out=vout[16:32], in_=vcm_v[16:32])

        # ---------------- pass A: s0_raw = sum_n u / 8 -----------------------
        sA = psum_s.tile([32, CE], F32, tag="s_ps")
        for q in range(NQ):
            wt = dma_w_quad(q)
            if q == 1:
                for h in range(4):  # xg (1MB) spread over 4 issues
                    nc.sync.dma_start(
                        out=xg_sb[:, 16 * h:16 * (h + 1), :],
                        in_=xg[:, 16 * h:16 * (h + 1), :])
            for gi in range(4):
                lhsT = xcA_sb[:, 4 * q + gi, :]
                for hh in range(4):
                    nc.tensor.matmul(
                        sA[:, hh * 512:(hh + 1) * 512],
                        lhsT,
                        wt[:, gi, hh * 512:(hh + 1) * 512],
                        start=(q == 0 and gi == 0),
                        stop=(q == NQ - 1 and gi == 3))
        s_to_v(sA, 0)

        # ---------------- passes B (it=1) and C (it=2) -----------------------
        for it in (1, 2):
            sP = psum_s.tile([32, CE], F32, tag="s_ps")
            vr = vrep[it - 1]
            t3q = []

            def flush_sel(ent, sP=sP):
                t3t, pair = ent
                for gg in range(2):
                    for hh in range(4):
                        nc.tensor.matmul(
                            sP[:, hh * 512:(hh + 1) * 512],
                            sel1_sb[:],
                            t3t[:, gg, hh * 512:(hh + 1) * 512],
                            start=(pair == 0 and gg == 0),
                            stop=(pair == 2 * NQ - 1 and gg == 1))

            def front(q, wt):
                u2p = [upool.tile([128, 2, CE], F16, name=f"u2_{pp}", tag="u2")
                       for pp in range(2)]
                for gi in range(4):
                    for h in range(2):
                        ups = psum_u.tile([128, 1024], F32, tag="ups")
                        for hh in range(2):
                            nc.tensor.matmul(
                                ups[:, hh * 512:(hh + 1) * 512],
                                xg_sb[:, 4 * q + gi, :],
                                wt[:, gi,
                                   h * 1024 + hh * 512:h * 1024 + (hh + 1) * 512],
                                start=True, stop=True)
                        dst = u2p[gi // 2][:, gi % 2, h * 1024:(h + 1) * 1024]
                        nc.scalar.copy(dst, ups[:])

                # db[p, g, c] = sum_e u*v: TT mult (2x) + top tree levels
                r1 = trpool.tile([128, 4, 1024], F16, tag="r1")
                for p in range(2):
                    t1 = t1pool.tile([128, 2, CE], F16, tag="t1")
                    nc.vector.tensor_mul(t1[:], u2p[p][:], _bcast(vr[:], 2, 0))
                    nc.vector.tensor_add(r1[:, 2 * p:2 * p + 2, :],
                                         t1[:, :, 0:1024], t1[:, :, 1024:2048])
                r2 = trpool.tile([128, 4, 512], F16, tag="r2")
                nc.vector.tensor_add(r2[:], r1[:, :, 0:512], r1[:, :, 512:1024])
                return u2p, r2

            def back_a(q, r2):
                # low tree levels + logit accumulation on Pool, then the
                # batched exp on ACT (right behind this quad's evacuations)
                r3 = trpool1.tile([128, 4, 256], F16, tag="r3")
                nc.gpsimd.tensor_add(r3[:], r2[:, :, 0:256], r2[:, :, 256:512])
                r4 = trpool1.tile([128, 4, 128], F16, tag="r4")
                nc.gpsimd.tensor_add(r4[:], r3[:, :, 0:128], r3[:, :, 128:256])
                if it == 1:
                    nc.gpsimd.tensor_add(bB[:, 4 * q:4 * q + 4, :],
                                         r4[:, :, 0:64], r4[:, :, 64:128])
                    blog = bB[:, 4 * q:4 * q + 4, :]
                else:
                    bt = smpool.tile([128, 4, CC], F16, tag="bt")
                    nc.gpsimd.tensor_add(bt[:], r4[:, :, 0:64], r4[:, :, 64:128])
                    bt2 = smpool.tile([128, 4, CC], F16, tag="bt2")
                    nc.gpsimd.tensor_add(bt2[:], bt[:], bB[:, 4 * q:4 * q + 4, :])
                    blog = bt2[:]
                eb = smpool.tile([128, 4, CC], F32, tag="eb")
                nc.scalar.activation(eb[:], blog[:], AF.Exp)
                return eb

            def back_b(q, u2p, eb):
                # softmax normalize + t3; emitted one quad late so DVE chews
                # the next quad's t1/r1/r2 while Pool/ACT finish this one
                zz = smpool.tile([128, 4], F32, tag="zz")
                nc.vector.tensor_reduce(zz[:], eb[:],
                                        axis=mybir.AxisListType.X, op=OP.add)
                iz = smpool.tile([128, 4], F32, tag="iz")
                nc.vector.reciprocal(iz[:], zz[:])
                cc = smpool.tile([128, 4, CC], F16, tag="cc")
                nc.vector.tensor_mul(cc[:], eb[:], _bcast(iz[:], CC, 1))
                # t3 = c*u (flat out/in0 + broadcast-AP in1 keeps DVE 2x)
                for p in range(2):
                    t3 = t3pool.tile([128, 2, CE], F16, tag="t3")
                    ccp = cc[:, 2 * p:2 * p + 2, :]
                    cc_b = bass.AP(tensor=ccp.tensor, offset=ccp.offset,
                                   ap=[list(ccp.ap[0]), list(ccp.ap[1]),
                                       [0, EE], list(ccp.ap[2])])
                    nc.vector.tensor_mul(t3[:], u2p[p][:], cc_b)
                    t3q.append((t3, 2 * q + p))
                    if len(t3q) > LAG:
                        flush_sel(t3q.pop(0))

            pend = None
            for q in range(NQ):
                wt = dma_w_quad(q)
                u2p, r2 = front(q, wt)
                eb = back_a(q, r2)
                if pend is not None:
                    back_b(*pend)
                pend = (q, u2p, eb)
            back_b(*pend)
            while t3q:
                flush_sel(t3q.pop(0))
            s_to_v(sP, it)

    _split_waits(nc)
    return nc


_CACHE = {}


def _prep_inputs(x, W, B):
    """Host-side fp16 layout prep: block-diagonal x tiles (ones bias row),
    collapsed x/8 for pass A (bias row 0.5), W rows=(j,d | bias=B) cols=(e,c)."""
    x = np.asarray(x, np.float32)
    W = np.asarray(W, np.float32)
    Bm = np.asarray(B, np.float32)

    xr = x.transpose(1, 2, 0).reshape(N_CORES, NG, G4, DD, BT)  # [c,g,j,d,b]
    xg = np.zeros((N_CORES, KK, NG, 128), np.float16)
    xcA = np.zeros((N_CORES, KK, NG, BT), np.float16)
    for j in range(G4):
        blk = xr[:, :, j].transpose(0, 2, 1, 3)  # [core, d, g, b]
        xg[:, j * DD:(j + 1) * DD, :, j * BT:(j + 1) * BT] = blk
        xcA[:, j * DD:(j + 1) * DD, :, :] = blk / 8.0
    xg[:, G4 * DD, :, :] = 1.0
    xcA[:, G4 * DD, :, :] = 0.5

    Wr = W.reshape(N_CORES, NG, G4, CC, DD, EE).transpose(0, 1, 2, 4, 5, 3)
    wg = np.zeros((N_CORES, NG, KK, CE), np.float16)
    wg[:, :, :G4 * DD, :] = Wr.reshape(N_CORES, NG, G4 * DD, CE)
    wg[:, :, G4 * DD, :] = Bm.T.reshape(CE).astype(np.float16)

    sel1 = np.zeros((128, BT), np.float16)
    for p in range(128):
        sel1[p, p % BT] = 1.0
    return xg, xcA, wg, sel1


def _in_maps(x, W, B):
    xg, xcA, wg, sel1 = _prep_inputs(x, W, B)
    return [
        {"xg": np.ascontiguousarray(xg[k]),
         "xcA": np.ascontiguousarray(xcA[k]),
         "wg": np.ascontiguousarray(wg[k]),
         "sel1": sel1}
        for k in range(N_CORES)
    ]


def kernel(x, W, B):
    if "nc" not in _CACHE:
        _CACHE["nc"] = _build_program()
    nc = _CACHE["nc"]
    res = run_bass_kernel_spmd(nc, _in_maps(x, W, B), list(range(N_CORES)))
    return np.asarray(res.results[0]["vout"], np.float32)


# revision 15
# speedup vs baseline: 1.2369x; 1.1619x over previous
"""CapsuleLayer (dynamic routing) Trainium2 kernel, 8-core SPMD.

Sharding: n_in (2048) split 8 ways -> 256 rows per core. W/x are sharded by n;
the only cross-core data is the [b, c, e] routing sum `s`, AllReduced once per
routing iteration (3x 256KB).

Device layout (per core):
  u[b, n, c, e] is produced by PE matmuls with K = (4n x 16d [+1 bias row]),
  M = 128 = (4 n-offsets x 32 batch), N = (e,c) chunks of 512. A partition row
  p of every on-chip [128, *] tensor is (j, b) = (p // 32, p % 32), i.e. n-local
  offset j within the 4-n group and batch b. The free axis of u is e-major
  (e*64 + c), which keeps every broadcast multiply at DVE 2x mode.

  Routing per iteration (streaming u from a DRAM fp16 scratch):
    db[p, c] = sum_e u * v   -> DVE mult + fp16 tree reduction over e
    softmax over c (free axis) -> ACT Exp with fused accum_out=Z, 1/Z on DVE
    s[b, (e,c)] += sum_n c * u -> DVE mult + PE matmul with a constant 0/1
      selector lhsT (sel[p, m] = p%32==m) contracting the partition axis per-b.
"""

import numpy as np
from contextlib import ExitStack

import concourse.bass as bass
import concourse.tile as tile
from concourse import mybir
from concourse.bass_utils import run_bass_kernel_spmd

F16 = mybir.dt.float16
F32 = mybir.dt.float32
AF = mybir.ActivationFunctionType
OP = mybir.AluOpType

N_CORES = 8
BT, NN, DD = 32, 2048, 16      # batch, n_in, d_in
CC, EE = 64, 32                # n_capsule, d_capsule
NL = NN // N_CORES             # 256 local n rows
G4 = 4                         # n rows per matmul group
NG = NL // G4                  # 64 groups
KK = G4 * DD + 1               # 65 contraction rows (incl. bias row)
CE = CC * EE                   # 2048, stored e-major: col = e*CC + c
EPS = 1e-9


def _split_waits(nc):
    """walrus CTRL codegen only supports one sem-wait per instruction; hoist
    extra waits into preceding NoOps on the same engine."""
    for f in nc.m.functions:
        for bb in f.blocks:
            new_insts = []
            for inst in bb.instructions:
                si = inst.sync_info
                if si is not None and si.on_wait and len(si.on_wait) > 1:
                    waits = list(si.on_wait)
                    for w in waits[:-1]:
                        new_insts.append(mybir.InstNoOp(
                            name=f"WS-{nc.next_id()}",
                            sync_info=mybir.SyncInfo(on_wait=[w], on_update=[]),
                            bass_nofuse=True,
                            engine=inst.engine,
                        ))
                    inst.sync_info = mybir.SyncInfo(
                        on_wait=waits[-1:], on_update=si.on_update)
                new_insts.append(inst)
            bb.instructions = new_insts


def _bcast(ap, n, axis_pos):
    """Insert a [step=0, count=n] dim into an AP at free-dim position axis_pos
    (0 = right after the partition dim)."""
    dims = [list(d) for d in ap.ap]
    dims.insert(1 + axis_pos, [0, n])
    return bass.AP(tensor=ap.tensor, offset=ap.offset, ap=dims)


def _build_program():
    nc = bass.Bass()
    xg = nc.declare_dram_parameter("xg", [KK, NG, 128], F16, isOutput=False)
    xc64 = nc.declare_dram_parameter("xc64", [KK, NG, 32], F16, isOutput=False)
    wg = nc.declare_dram_parameter("wg", [NG, KK, CE], F16, isOutput=False)
    sel64 = nc.declare_dram_parameter("sel64", [128, 32], F16, isOutput=False)
    sel1 = nc.declare_dram_parameter("sel1", [128, 32], F16, isOutput=False)
    vout = nc.declare_dram_parameter("vout", [BT, CC, EE], F32, isOutput=True)

    with ExitStack() as ctx:
        tc = ctx.enter_context(tile.TileContext(nc))
        singles = ctx.enter_context(tc.tile_pool(name="singles", bufs=1))
        wpool = ctx.enter_context(tc.tile_pool(name="wpool", bufs=3))
        upool = ctx.enter_context(tc.tile_pool(name="upool", bufs=5))
        tpool = ctx.enter_context(tc.tile_pool(name="tpool", bufs=2))
        t3pool = ctx.enter_context(tc.tile_pool(name="t3pool", bufs=5))
        trpool = ctx.enter_context(tc.tile_pool(name="trpool", bufs=2))
        smpool = ctx.enter_context(tc.tile_pool(name="smpool", bufs=2))
        vpool = ctx.enter_context(tc.tile_pool(name="vpool", bufs=1))
        psum_u = ctx.enter_context(tc.tile_pool(name="psum_u", bufs=2, space="PSUM"))
        psum_s = ctx.enter_context(tc.tile_pool(name="psum_s", bufs=1, space="PSUM"))
        dram = ctx.enter_context(tc.tile_pool(name="dram", bufs=1, space="DRAM"))

        xg_sb = singles.tile([KK, NG, 128], F16)
        nc.sync.dma_start(out=xg_sb[:], in_=xg[:])
        xc64_sb = singles.tile([KK, NG, 32], F16)
        nc.sync.dma_start(out=xc64_sb[:], in_=xc64[:])
        sel64_sb = singles.tile([128, 32], F16)
        nc.sync.dma_start(out=sel64_sb[:], in_=sel64[:])
        sel1_sb = singles.tile([128, 32], F16)
        nc.sync.dma_start(out=sel1_sb[:], in_=sel1[:])

        bB = singles.tile([128, NG, CC], F16)       # logits b after pass B
        vrep = [singles.tile([128, CE], F16, name="vrep0", tag="vrep0"),
                singles.tile([128, CE], F16, name="vrep1", tag="vrep1")]

        def s_to_v(s_ps, it):
            """Evacuate the s psum, AllReduce across cores, squash -> v.
            Builds vrep[it] (fp16, partition-replicated x4); for the last
            iteration writes vout instead."""
            s_sb = vpool.tile([32, CE], F16, tag="s_sb")
            nc.vector.tensor_copy(s_sb[:], s_ps[:])
            sloc = dram.tile([32, CE], F16, tag=f"sloc{it}")
            for q in range(2):
                nc.scalar.dma_start(out=sloc[:, q * 1024:(q + 1) * 1024],
                                    in_=s_sb[:, q * 1024:(q + 1) * 1024])
            ssum = dram.tile([32, CE], F16, tag=f"ssum{it}")
            nc.gpsimd.collective_compute(
                "AllReduce", OP.add,
                replica_groups=[list(range(N_CORES))],
                ins=[sloc[:].opt()], outs=[ssum[:].opt()])
            ssb = vpool.tile([32, CE], F16, tag="ssb")
            for q in range(2):
                nc.scalar.dma_start(out=ssb[:, q * 1024:(q + 1) * 1024],
                                    in_=ssum[:, q * 1024:(q + 1) * 1024])

            # squash scale = ns/(1+ns)/sqrt(ns+eps), ns = sum_e s^2  [32, C]
            s2 = vpool.tile([32, CE], F16, tag="s2")
            nc.vector.tensor_mul(s2[:], ssb[:], ssb[:])
            s2v = s2[:].rearrange("p (e c) -> p c e", e=EE)
            ns = smpool.tile([32, CC], F32, tag="ns")
            nc.vector.tensor_reduce(ns[:], s2v, axis=mybir.AxisListType.X, op=OP.add)
            sq = smpool.tile([32, CC], F32, tag="sq")
            epst = smpool.tile([32, 1], F32, tag="epst")
            nc.vector.memset(epst[:], EPS)
            nc.scalar.activation(sq[:], ns[:], AF.Sqrt, bias=epst[:], scale=1.0)
            den = smpool.tile([32, CC], F32, tag="den")
            nc.vector.scalar_tensor_tensor(den[:], ns[:], 1.0, sq[:],
                                           op0=OP.add, op1=OP.mult)
            inv = smpool.tile([32, CC], F32, tag="inv")
            nc.vector.reciprocal(inv[:], den[:])
            scale = smpool.tile([32, CC], F32, tag="scale")
            nc.vector.tensor_mul(scale[:], ns[:], inv[:])

            if it == 2:
                # v = s*scale, written through a transposed AP so the DMA-out
                # sees contiguous [b, c, e]
                vcm = vpool.tile([32, CE], F32, tag="vcm")
                vcm_t = bass.AP(
                    tensor=vcm[:].tensor, offset=vcm[:].offset,
                    ap=[list(vcm[:].ap[0]), [1, EE], [EE, CC]])
                nc.vector.tensor_mul(vcm_t, ssb[:], _bcast(scale[:], EE, 0))
                vcm_v = vcm[:].rearrange("p (c e) -> p c e", c=CC)
                nc.scalar.dma_start(out=vout[0:16], in_=vcm_v[0:16])
                nc.scalar.dma_start(out=vout[16:32], in_=vcm_v[16:32])
                return

            # replicate s (concurrent with the squash chain) and scale across
            # the 4 partition groups via DRAM, then one multiply into vrep
            scd = dram.tile([32, CC], F32, tag=f"scd{it}")
            nc.scalar.dma_start(out=scd[:], in_=scale[:])
            screp = smpool.tile([128, CC], F32, tag="screp")
            scd_ap = scd[:]
            rep_sc = bass.AP(tensor=scd_ap.tensor, offset=scd_ap.offset,
                             ap=[[0, 4]] + [list(d) for d in scd_ap.ap])
            nc.scalar.dma_start(out=screp[:], in_=rep_sc)
            srep = vpool.tile([128, CE], F16, tag="srep")
            for q in range(2):
                half = ssum[:, q * 1024:(q + 1) * 1024]
                rep_s = bass.AP(tensor=half.tensor, offset=half.offset,
                                ap=[[0, 4]] + [list(d) for d in half.ap])
                nc.scalar.dma_start(out=srep[:, q * 1024:(q + 1) * 1024], in_=rep_s)
            nc.vector.tensor_mul(vrep[it][:], srep[:], _bcast(screp[:], EE, 0))

        # ---------------- pass A: s0 = sum_n (u+B) / 64 directly from W ------
        sA = psum_s.tile([32, CE], F32, tag="s_ps")
        for gp in range(NG // 2):
            wt = wpool.tile([KK, 2, CE], F16, tag="wt2")
            for gg in range(2):
                eng = nc.sync if gg == 0 else nc.gpsimd
                for q in range(2):
                    lo = q * 1024
                    eng.dma_start(out=wt[:, gg, lo:lo + 1024],
                                  in_=wg[2 * gp + gg, :, lo:lo + 1024])
            for gg in range(2):
                for q in range(4):
                    nc.tensor.matmul(
                        sA[:, q * 512:(q + 1) * 512],
                        xc64_sb[:, 2 * gp + gg, :],
                        wt[:, gg, q * 512:(q + 1) * 512],
                        start=(gp == 0 and gg == 0),
                        stop=(gp == NG // 2 - 1 and gg == 1))
        s_to_v(sA, 0)

        # ---------------- passes B (it=1) and C (it=2) -----------------------
        # u is recomputed on the fly (PE, 8-matmul bursts per group pair keep
        # the HAM clock-gate open) instead of streamed from DRAM; psum
        # evacuation rides on ScalarE; all big DVE ops cover a group PAIR to
        # amortize the per-op pipeline overhead; sel-matmuls are emitted one
        # pair late so the PE FIFO never blocks on the current pair's DVE.
        for it in (1, 2):
            sP = psum_s.tile([32, CE], F32, tag="s_ps")
            vr = vrep[it - 1]
            t3_q = []

            def flush_t3(t3p, first, last):
                # start/stop are per psum bank (q-slice)
                for gg in range(2):
                    for q in range(4):
                        nc.tensor.matmul(
                            sP[:, q * 512:(q + 1) * 512],
                            sel1_sb[:],
                            t3p[:, gg, q * 512:(q + 1) * 512],
                            start=(first and gg == 0),
                            stop=(last and gg == 1))

            for gp in range(NG // 2):
                wt = wpool.tile([KK, 2, CE], F16, tag="wt2")
                for gg in range(2):
                    eng = nc.sync if gg == 0 else nc.gpsimd
                    for q in range(2):
                        lo = q * 1024
                        eng.dma_start(
                            out=wt[:, gg, lo:lo + 1024],
                            in_=wg[2 * gp + gg, :, lo:lo + 1024])
                u2 = upool.tile([128, 2, CE], F16, tag="u_full")
                for gg in range(2):
                    for h in range(2):
                        ups = psum_u.tile([128, 1024], F32, tag="ups")
                        for q in range(2):
                            nc.tensor.matmul(
                                ups[:, q * 512:(q + 1) * 512],
                                xg_sb[:, 2 * gp + gg, :],
                                wt[:, gg, h * 1024 + q * 512:
                                   h * 1024 + (q + 1) * 512],
                                start=True, stop=True)
                        nc.scalar.copy(u2[:, gg, h * 1024:(h + 1) * 1024],
                                       ups[:])
                # db = sum_e u*v : fp16 mult + fp16 tree over e (e-major)
                t1 = tpool.tile([128, 2, CE], F16, tag="t1")
                nc.vector.tensor_mul(t1[:], u2[:], _bcast(vr[:], 2, 0))
                t1v = t1[:].rearrange("p g (e c) -> p g e c", e=EE)
                r1 = trpool.tile([128, 2, 16, CC], F16, tag="r1")
                nc.vector.tensor_add(r1[:], t1v[:, :, 0:16, :], t1v[:, :, 16:32, :])
                r2 = trpool.tile([128, 2, 8, CC], F16, tag="r2")
                nc.vector.tensor_add(r2[:], r1[:, :, 0:8, :], r1[:, :, 8:16, :])
                r3 = trpool.tile([128, 2, 4, CC], F16, tag="r3")
                nc.vector.tensor_add(r3[:], r2[:, :, 0:4, :], r2[:, :, 4:8, :])
                r4 = trpool.tile([128, 2, 2, CC], F16, tag="r4")
                nc.vector.tensor_add(r4[:], r3[:, :, 0:2, :], r3[:, :, 2:4, :])
                if it == 1:
                    blog = bB[:, 2 * gp:2 * gp + 2, :]
                    nc.vector.tensor_add(blog, r4[:, :, 0, :], r4[:, :, 1, :])
                else:
                    bt = smpool.tile([128, 2, CC], F32, tag="bt")
                    nc.vector.tensor_add(bt[:], r4[:, :, 0, :], r4[:, :, 1, :])
                    bt2 = smpool.tile([128, 2, CC], F32, tag="bt2")
                    nc.vector.tensor_add(bt2[:], bt[:], bB[:, 2 * gp:2 * gp + 2, :])
                    blog = bt2[:]
                # softmax over c (free axis); Z must stay per-group
                cc = smpool.tile([128, 2, CC], F16, tag="cc")
                for gg in range(2):
                    eb = smpool.tile([128, CC], F32, tag="eb")
                    zz = smpool.tile([128, 1], F32, tag="zz")
                    nc.scalar.activation(eb[:], blog[:, gg, :], AF.Exp,
                                         accum_out=zz[:])
                    iz = smpool.tile([128, 1], F32, tag="iz")
                    nc.vector.reciprocal(iz[:], zz[:])
                    nc.vector.tensor_scalar_mul(cc[:, gg, :], eb[:], iz[:])
                # s += sum_n c*u
                t3 = t3pool.tile([128, 2, CE], F16, tag="t3")
                cc_ap = cc[:]
                cc_b = bass.AP(tensor=cc_ap.tensor, offset=cc_ap.offset,
                               ap=[list(cc_ap.ap[0]), list(cc_ap.ap[1]),
                                   [0, EE], list(cc_ap.ap[2])])
                nc.vector.tensor_mul(t3[:], u2[:], cc_b)
                t3_q.append((t3, gp))
                if len(t3_q) > 4:
                    tt, gpp = t3_q.pop(0)
                    flush_t3(tt, first=(gpp == 0), last=False)
            while t3_q:
                tt, gpp = t3_q.pop(0)
                flush_t3(tt, first=(gpp == 0), last=(len(t3_q) == 0))
            s_to_v(sP, it)

    _split_waits(nc)
    return nc


_CACHE = {}


def _prep_inputs(x, W, B):
    """Host-side layout prep: fp16 casts, n-sharding, block-diagonal x tiles
    (with a ones row for the bias), W permuted to rows=(j,d) cols=(e,c)."""
    x = np.asarray(x, np.float32)
    W = np.asarray(W, np.float32)
    Bmat = np.asarray(B, np.float32)

    # xg[core, k=(j*16+d | 64), nG, m=(j*32+b)]
    xg = np.zeros((N_CORES, KK, NG, 128), np.float16)
    xr = x.transpose(1, 2, 0).reshape(N_CORES, NG, G4, DD, BT)  # [core,g,j,d,b]
    for j in range(G4):
        xg[:, j * DD:(j + 1) * DD, :, j * BT:(j + 1) * BT] = \
            xr[:, :, j].transpose(0, 2, 1, 3)
    xg[:, G4 * DD, :, :] = 1.0

    # collapsed 1/64-scaled x for the direct s0 matmul: dense columns (M=b),
    # rows = all (j, d) pairs; bias row 4/64 (4 n-rows per group, each +B)
    xc64 = np.zeros((N_CORES, KK, NG, 32), np.float16)
    xc64[:, :G4 * DD] = (xr.transpose(0, 2, 3, 1, 4) / NG
                         ).reshape(N_CORES, G4 * DD, NG, BT).astype(np.float16)
    xc64[:, G4 * DD] = G4 / NG

    # wg[core, g, k, e*64+c]
    wg = np.zeros((N_CORES, NG, KK, CE), np.float16)
    Wr = W.reshape(N_CORES, NG, G4, CC, DD, EE).transpose(0, 1, 2, 4, 5, 3)
    wg[:, :, :G4 * DD, :] = Wr.reshape(N_CORES, NG, G4 * DD, CE)
    wg[:, :, G4 * DD, :] = Bmat.T.reshape(CE).astype(np.float16)

    sel64 = np.zeros((128, 32), np.float16)
    sel1 = np.zeros((128, 32), np.float16)
    for p in range(128):
        sel64[p, p % 32] = 1.0 / NG
        sel1[p, p % 32] = 1.0
    return xg, xc64, wg, sel64, sel1


def _in_maps(x, W, B):
    xg, xc64, wg, sel64, sel1 = _prep_inputs(x, W, B)
    return [
        {"xg": np.ascontiguousarray(xg[k]),
         "xc64": np.ascontiguousarray(xc64[k]),
         "wg": np.ascontiguousarray(wg[k]),
         "sel64": sel64, "sel1": sel1}
        for k in range(N_CORES)
    ]


def kernel(x, W, B):
    if "nc" not in _CACHE:
        _CACHE["nc"] = _build_program()
    nc = _CACHE["nc"]
    res = run_bass_kernel_spmd(nc, _in_maps(x, W, B), list(range(N_CORES)))
    return np.asarray(res.results[0]["vout"], np.float32)



# revision 16
# speedup vs baseline: 1.2552x; 1.0149x over previous
"""CapsuleLayer (dynamic routing) Trainium2 kernel, 8-core SPMD.

Sharding: n_in (2048) split 8 ways -> 256 rows per core. W/x are sharded by n;
the only cross-core data is the [b, c, e] routing sum `s`, AllReduced once per
routing iteration (3x 256KB).

Device layout (per core):
  u[b, n, c, e] is produced by PE matmuls with K = (4n x 16d [+1 bias row]),
  M = 128 = (4 n-offsets x 32 batch), N = (e,c) chunks of 512. A partition row
  p of every on-chip [128, *] tensor is (j, b) = (p // 32, p % 32), i.e. n-local
  offset j within the 4-n group and batch b. The free axis of u is e-major
  (e*64 + c), which keeps every broadcast multiply at DVE 2x mode.

  Routing per iteration (streaming u from a DRAM fp16 scratch):
    db[p, c] = sum_e u * v   -> DVE mult + fp16 tree reduction over e
    softmax over c (free axis) -> ACT Exp with fused accum_out=Z, 1/Z on DVE
    s[b, (e,c)] += sum_n c * u -> DVE mult + PE matmul with a constant 0/1
      selector lhsT (sel[p, m] = p%32==m) contracting the partition axis per-b.
"""

import numpy as np
from contextlib import ExitStack

import concourse.bass as bass
import concourse.tile as tile
from concourse import mybir
from concourse.bass_utils import run_bass_kernel_spmd

F16 = mybir.dt.float16
F32 = mybir.dt.float32
AF = mybir.ActivationFunctionType
OP = mybir.AluOpType

N_CORES = 8
BT, NN, DD = 32, 2048, 16      # batch, n_in, d_in
CC, EE = 64, 32                # n_capsule, d_capsule
NL = NN // N_CORES             # 256 local n rows
G4 = 4                         # n rows per matmul group
NG = NL // G4                  # 64 groups
KK = G4 * DD + 1               # 65 contraction rows (incl. bias row)
CE = CC * EE                   # 2048, stored e-major: col = e*CC + c
EPS = 1e-9


def _split_waits(nc):
    """walrus CTRL codegen only supports one sem-wait per instruction; hoist
    extra waits into preceding NoOps on the same engine."""
    for f in nc.m.functions:
        for bb in f.blocks:
            new_insts = []
            for inst in bb.instructions:
                si = inst.sync_info
                if si is not None and si.on_wait and len(si.on_wait) > 1:
                    waits = list(si.on_wait)
                    for w in waits[:-1]:
                        new_insts.append(mybir.InstNoOp(
                            name=f"WS-{nc.next_id()}",
                            sync_info=mybir.SyncInfo(on_wait=[w], on_update=[]),
                            bass_nofuse=True,
                            engine=inst.engine,
                        ))
                    inst.sync_info = mybir.SyncInfo(
                        on_wait=waits[-1:], on_update=si.on_update)
                new_insts.append(inst)
            bb.instructions = new_insts


def _bcast(ap, n, axis_pos):
    """Insert a [step=0, count=n] dim into an AP at free-dim position axis_pos
    (0 = right after the partition dim)."""
    dims = [list(d) for d in ap.ap]
    dims.insert(1 + axis_pos, [0, n])
    return bass.AP(tensor=ap.tensor, offset=ap.offset, ap=dims)


def _build_program():
    nc = bass.Bass()
    xg = nc.declare_dram_parameter("xg", [KK, NG, 128], F16, isOutput=False)
    xc64 = nc.declare_dram_parameter("xc64", [KK, NG, 32], F16, isOutput=False)
    wg = nc.declare_dram_parameter("wg", [NG, KK, CE], F16, isOutput=False)
    sel64 = nc.declare_dram_parameter("sel64", [128, 32], F16, isOutput=False)
    sel1 = nc.declare_dram_parameter("sel1", [128, 32], F16, isOutput=False)
    vout = nc.declare_dram_parameter("vout", [BT, CC, EE], F32, isOutput=True)

    with ExitStack() as ctx:
        tc = ctx.enter_context(tile.TileContext(nc))
        singles = ctx.enter_context(tc.tile_pool(name="singles", bufs=1))
        wpool = ctx.enter_context(tc.tile_pool(name="wpool", bufs=5))
        upool = ctx.enter_context(tc.tile_pool(name="upool", bufs=5))
        tpool = ctx.enter_context(tc.tile_pool(name="tpool", bufs=2))
        trpool = ctx.enter_context(tc.tile_pool(name="trpool", bufs=2))
        smpool = ctx.enter_context(tc.tile_pool(name="smpool", bufs=2))
        vpool = ctx.enter_context(tc.tile_pool(name="vpool", bufs=1))
        psum_u = ctx.enter_context(tc.tile_pool(name="psum_u", bufs=2, space="PSUM"))
        psum_s = ctx.enter_context(tc.tile_pool(name="psum_s", bufs=1, space="PSUM"))
        dram = ctx.enter_context(tc.tile_pool(name="dram", bufs=1, space="DRAM"))

        xg_sb = singles.tile([KK, NG, 128], F16)
        nc.sync.dma_start(out=xg_sb[:], in_=xg[:])
        xc64_sb = singles.tile([KK, NG, 32], F16)
        nc.sync.dma_start(out=xc64_sb[:], in_=xc64[:])
        sel64_sb = singles.tile([128, 32], F16)
        nc.sync.dma_start(out=sel64_sb[:], in_=sel64[:])
        sel1_sb = singles.tile([128, 32], F16)
        nc.sync.dma_start(out=sel1_sb[:], in_=sel1[:])

        bB = singles.tile([128, NG, CC], F32)       # logits b after pass B
        vrep = [singles.tile([128, CE], F16, name="vrep0", tag="vrep0"),
                singles.tile([128, CE], F16, name="vrep1", tag="vrep1")]

        def s_to_v(s_ps, it):
            """Evacuate the s psum, AllReduce across cores, squash -> v.
            Builds vrep[it] (fp16, partition-replicated x4); for the last
            iteration writes vout instead."""
            s_sb = vpool.tile([32, CE], F16, tag="s_sb")
            nc.vector.tensor_copy(s_sb[:], s_ps[:])
            sloc = dram.tile([32, CE], F16, tag=f"sloc{it}")
            for q in range(2):
                nc.sync.dma_start(out=sloc[:, q * 1024:(q + 1) * 1024],
                                  in_=s_sb[:, q * 1024:(q + 1) * 1024])
            ssum = dram.tile([32, CE], F16, tag=f"ssum{it}")
            nc.gpsimd.collective_compute(
                "AllReduce", OP.add,
                replica_groups=[list(range(N_CORES))],
                ins=[sloc[:].opt()], outs=[ssum[:].opt()])
            ssb = vpool.tile([32, CE], F16, tag="ssb")
            for q in range(2):
                nc.sync.dma_start(out=ssb[:, q * 1024:(q + 1) * 1024],
                                  in_=ssum[:, q * 1024:(q + 1) * 1024])

            # squash scale = ns/(1+ns)/sqrt(ns+eps), ns = sum_e s^2  [32, C]
            s2 = vpool.tile([32, CE], F16, tag="s2")
            nc.vector.tensor_mul(s2[:], ssb[:], ssb[:])
            s2v = s2[:].rearrange("p (e c) -> p c e", e=EE)
            ns = smpool.tile([32, CC], F32, tag="ns")
            nc.vector.tensor_reduce(ns[:], s2v, axis=mybir.AxisListType.X, op=OP.add)
            sq = smpool.tile([32, CC], F32, tag="sq")
            epst = smpool.tile([32, 1], F32, tag="epst")
            nc.vector.memset(epst[:], EPS)
            nc.scalar.activation(sq[:], ns[:], AF.Sqrt, bias=epst[:], scale=1.0)
            den = smpool.tile([32, CC], F32, tag="den")
            nc.vector.scalar_tensor_tensor(den[:], ns[:], 1.0, sq[:],
                                           op0=OP.add, op1=OP.mult)
            inv = smpool.tile([32, CC], F32, tag="inv")
            nc.vector.reciprocal(inv[:], den[:])
            scale = smpool.tile([32, CC], F32, tag="scale")
            nc.vector.tensor_mul(scale[:], ns[:], inv[:])

            if it == 2:
                # v = s*scale, written through a transposed AP so the DMA-out
                # sees contiguous [b, c, e]
                vcm = vpool.tile([32, CE], F32, tag="vcm")
                vcm_t = bass.AP(
                    tensor=vcm[:].tensor, offset=vcm[:].offset,
                    ap=[list(vcm[:].ap[0]), [1, EE], [EE, CC]])
                nc.vector.tensor_mul(vcm_t, ssb[:], _bcast(scale[:], EE, 0))
                vcm_v = vcm[:].rearrange("p (c e) -> p c e", c=CC)
                nc.sync.dma_start(out=vout[0:16], in_=vcm_v[0:16])
                nc.sync.dma_start(out=vout[16:32], in_=vcm_v[16:32])
                return

            # replicate s (concurrent with the squash chain) and scale across
            # the 4 partition groups via DRAM, then one multiply into vrep
            scd = dram.tile([32, CC], F32, tag=f"scd{it}")
            nc.sync.dma_start(out=scd[:], in_=scale[:])
            screp = smpool.tile([128, CC], F32, tag="screp")
            scd_ap = scd[:]
            rep_sc = bass.AP(tensor=scd_ap.tensor, offset=scd_ap.offset,
                             ap=[[0, 4]] + [list(d) for d in scd_ap.ap])
            nc.sync.dma_start(out=screp[:], in_=rep_sc)
            srep = vpool.tile([128, CE], F16, tag="srep")
            for q in range(2):
                half = ssum[:, q * 1024:(q + 1) * 1024]
                rep_s = bass.AP(tensor=half.tensor, offset=half.offset,
                                ap=[[0, 4]] + [list(d) for d in half.ap])
                nc.sync.dma_start(out=srep[:, q * 1024:(q + 1) * 1024], in_=rep_s)
            nc.vector.tensor_mul(vrep[it][:], srep[:], _bcast(screp[:], EE, 0))

        # ---------------- pass A: s0 = sum_n (u+B) / 64 directly from W ------
        sA = psum_s.tile([32, CE], F32, tag="s_ps")
        for gp in range(NG // 2):
            wt = wpool.tile([KK, 2, CE], F16, tag="wt2")
            for gg in range(2):
                eng = nc.sync if gg == 0 else nc.gpsimd
                for q in range(2):
                    lo = q * 1024
                    eng.dma_start(out=wt[:, gg, lo:lo + 1024],
                                  in_=wg[2 * gp + gg, :, lo:lo + 1024])
            for gg in range(2):
                for q in range(4):
                    nc.tensor.matmul(
                        sA[:, q * 512:(q + 1) * 512],
                        xc64_sb[:, 2 * gp + gg, :],
                        wt[:, gg, q * 512:(q + 1) * 512],
                        start=(gp == 0 and gg == 0),
                        stop=(gp == NG // 2 - 1 and gg == 1))
        s_to_v(sA, 0)

        # ---------------- passes B (it=1) and C (it=2) -----------------------
        # u is recomputed on the fly (PE, 8-matmul bursts per group pair keep
        # the HAM clock-gate open) instead of streamed from DRAM; psum
        # evacuation rides on ScalarE; all big DVE ops cover a group PAIR to
        # amortize the per-op pipeline overhead; sel-matmuls are emitted one
        # pair late so the PE FIFO never blocks on the current pair's DVE.
        for it in (1, 2):
            sP = psum_s.tile([32, CE], F32, tag="s_ps")
            vr = vrep[it - 1]
            t3_q = []

            def flush_t3(t3p, first, last):
                # start/stop are per psum bank (q-slice)
                for gg in range(2):
                    for q in range(4):
                        nc.tensor.matmul(
                            sP[:, q * 512:(q + 1) * 512],
                            sel1_sb[:],
                            t3p[:, gg, q * 512:(q + 1) * 512],
                            start=(first and gg == 0),
                            stop=(last and gg == 1))

            for gp in range(NG // 2):
                wt = wpool.tile([KK, 2, CE], F16, tag="wt2")
                for gg in range(2):
                    eng = nc.sync if gg == 0 else nc.gpsimd
                    for q in range(2):
                        lo = q * 1024
                        eng.dma_start(
                            out=wt[:, gg, lo:lo + 1024],
                            in_=wg[2 * gp + gg, :, lo:lo + 1024])
                u2 = upool.tile([128, 2, CE], F16, tag="u_full")
                for gg in range(2):
                    for h in range(2):
                        ups = psum_u.tile([128, 1024], F32, tag="ups")
                        for q in range(2):
                            nc.tensor.matmul(
                                ups[:, q * 512:(q + 1) * 512],
                                xg_sb[:, 2 * gp + gg, :],
                                wt[:, gg, h * 1024 + q * 512:
                                   h * 1024 + (q + 1) * 512],
                                start=True, stop=True)
                        nc.scalar.copy(u2[:, gg, h * 1024:(h + 1) * 1024],
                                       ups[:])
                # db = sum_e u*v : fp16 mult + fp16 tree over e (e-major)
                t1 = tpool.tile([128, 2, CE], F16, tag="t1")
                nc.vector.tensor_mul(t1[:], u2[:], _bcast(vr[:], 2, 0))
                t1v = t1[:].rearrange("p g (e c) -> p g e c", e=EE)
                r1 = trpool.tile([128, 2, 16, CC], F16, tag="r1")
                nc.vector.tensor_add(r1[:], t1v[:, :, 0:16, :], t1v[:, :, 16:32, :])
                r2 = trpool.tile([128, 2, 8, CC], F16, tag="r2")
                nc.vector.tensor_add(r2[:], r1[:, :, 0:8, :], r1[:, :, 8:16, :])
                r3 = trpool.tile([128, 2, 4, CC], F16, tag="r3")
                nc.vector.tensor_add(r3[:], r2[:, :, 0:4, :], r2[:, :, 4:8, :])
                r4 = trpool.tile([128, 2, 2, CC], F16, tag="r4")
                nc.vector.tensor_add(r4[:], r3[:, :, 0:2, :], r3[:, :, 2:4, :])
                if it == 1:
                    blog = bB[:, 2 * gp:2 * gp + 2, :]
                    nc.vector.tensor_add(blog, r4[:, :, 0, :], r4[:, :, 1, :])
                else:
                    bt = smpool.tile([128, 2, CC], F32, tag="bt")
                    nc.vector.tensor_add(bt[:], r4[:, :, 0, :], r4[:, :, 1, :])
                    bt2 = smpool.tile([128, 2, CC], F32, tag="bt2")
                    nc.vector.tensor_add(bt2[:], bt[:], bB[:, 2 * gp:2 * gp + 2, :])
                    blog = bt2[:]
                # softmax over c (free axis); Z must stay per-group
                cc = smpool.tile([128, 2, CC], F16, tag="cc")
                for gg in range(2):
                    eb = smpool.tile([128, CC], F32, tag="eb")
                    zz = smpool.tile([128, 1], F32, tag="zz")
                    nc.scalar.activation(eb[:], blog[:, gg, :], AF.Exp,
                                         accum_out=zz[:])
                    iz = smpool.tile([128, 1], F32, tag="iz")
                    nc.vector.reciprocal(iz[:], zz[:])
                    nc.vector.tensor_scalar_mul(cc[:, gg, :], eb[:], iz[:])
                # s += sum_n c*u
                t3 = tpool.tile([128, 2, CE], F16, tag="t3")
                cc_ap = cc[:]
                cc_b = bass.AP(tensor=cc_ap.tensor, offset=cc_ap.offset,
                               ap=[list(cc_ap.ap[0]), list(cc_ap.ap[1]),
                                   [0, EE], list(cc_ap.ap[2])])
                nc.vector.tensor_mul(t3[:], u2[:], cc_b)
                t3_q.append(t3)
                if len(t3_q) > 1:
                    flush_t3(t3_q.pop(0), first=(gp == 1), last=False)
            flush_t3(t3_q.pop(0), first=False, last=True)
            s_to_v(sP, it)

    _split_waits(nc)
    return nc


_CACHE = {}


def _prep_inputs(x, W, B):
    """Host-side layout prep: fp16 casts, n-sharding, block-diagonal x tiles
    (with a ones row for the bias), W permuted to rows=(j,d) cols=(e,c)."""
    x = np.asarray(x, np.float32)
    W = np.asarray(W, np.float32)
    Bmat = np.asarray(B, np.float32)

    # xg[core, k=(j*16+d | 64), nG, m=(j*32+b)]
    xg = np.zeros((N_CORES, KK, NG, 128), np.float16)
    xr = x.transpose(1, 2, 0).reshape(N_CORES, NG, G4, DD, BT)  # [core,g,j,d,b]
    for j in range(G4):
        xg[:, j * DD:(j + 1) * DD, :, j * BT:(j + 1) * BT] = \
            xr[:, :, j].transpose(0, 2, 1, 3)
    xg[:, G4 * DD, :, :] = 1.0

    # collapsed 1/64-scaled x for the direct s0 matmul: dense columns (M=b),
    # rows = all (j, d) pairs; bias row 4/64 (4 n-rows per group, each +B)
    xc64 = np.zeros((N_CORES, KK, NG, 32), np.float16)
    xc64[:, :G4 * DD] = (xr.transpose(0, 2, 3, 1, 4) / NG
                         ).reshape(N_CORES, G4 * DD, NG, BT).astype(np.float16)
    xc64[:, G4 * DD] = G4 / NG

    # wg[core, g, k, e*64+c]
    wg = np.zeros((N_CORES, NG, KK, CE), np.float16)
    Wr = W.reshape(N_CORES, NG, G4, CC, DD, EE).transpose(0, 1, 2, 4, 5, 3)
    wg[:, :, :G4 * DD, :] = Wr.reshape(N_CORES, NG, G4 * DD, CE)
    wg[:, :, G4 * DD, :] = Bmat.T.reshape(CE).astype(np.float16)

    sel64 = np.zeros((128, 32), np.float16)
    sel1 = np.zeros((128, 32), np.float16)
    for p in range(128):
        sel64[p, p % 32] = 1.0 / NG
        sel1[p, p % 32] = 1.0
    return xg, xc64, wg, sel64, sel1


def _in_maps(x, W, B):
    xg, xc64, wg, sel64, sel1 = _prep_inputs(x, W, B)
    return [
        {"xg": np.ascontiguousarray(xg[k]),
         "xc64": np.ascontiguousarray(xc64[k]),
         "wg": np.ascontiguousarray(wg[k]),
         "sel64": sel64, "sel1": sel1}
        for k in range(N_CORES)
    ]


def kernel(x, W, B):
    if "nc" not in _CACHE:
        _CACHE["nc"] = _build_program()
    nc = _CACHE["nc"]
    res = run_bass_kernel_spmd(nc, _in_maps(x, W, B), list(range(N_CORES)))
    return np.asarray(res.results[0]["vout"], np.float32)

